# revision 3
# baseline (speedup 1.0000x reference)
"""Trainium2 Bass kernel for the fuzzy joint-membership layer.

Math (derived from the reference 2-qubit circuit, verified vs oracle):
  out[b, 2p,   c] = 0.5 + 0.5*cos(theta_c)*cos(x0) - 0.5*sin(theta_c)*sin(x0)*sin(x1)
  out[b, 2p+1, c] = 0.5 + 0.5*cos(x0)*cos(x1)
where x0 = xf[b, pair_idx[b,p,0]], x1 = xf[b, pair_idx[b,p,1]].

Sharding: pure data parallel, batch 4096 -> 8 cores x 512 rows.

Gather strategy: the per-row gather xf[b, idx[b,j]] is expressed as
gpsimd local_scatter (hardware vector scatter in Q7 local RAM, with
per-partition independent indices) instead of ap_gather (which costs
~36 cycles per index on the Q7 command interface):
  - host precomputes, per row, for each pixel the FIRST slot j wanting
    it (idxA[row, pix] = j or -1) plus log-doubling duplicate maps
    CT[t][row, src_slot] = dst_slot covering ordinals [2^t, 2^{t+1})
  - S0 = scatter(x16, idxA); U = S0
  - round t: St = scatter(U, CT[t]); U = U + St (disjoint, exact fp16)
x is moved in fp16 (abs err <= ~1.5e-3 after trig, vs 2e-2 tolerance).

Slot layout is half-split (x0 of pair p -> slot p, x1 -> slot 460+p)
so all downstream pair reads are unit-stride (DVE 2x fp16 mode).
Intermediates and the output tile are fp16 (host upcasts to f32;
error budget ~4e-3 vs the 2e-2 gate). Range reduction (magic round)
runs on ACT, Sin on ACT, products + class expansion on DVE with
broadcast APs, odd block replicated by one broadcast ACT copy.
"""

import math
import numpy as np

B, PIX, NPAIR, C = 4096, 3072, 460, 10
NG = 2 * NPAIR          # 920 gathered values per row
OUTW = NG * C           # 9200
NCORES = 8
BS = B // NCORES        # 512 rows per core
TILES = BS // 128       # 4

_cache = {}


def _ensure_path():
    try:
        import concourse  # noqa: F401
    except ImportError:
        import sys
        sys.path.insert(0, "/opt/trn_rl_repo")


def build_nc(bs=BS, rounds=3):
    _ensure_path()
    from contextlib import ExitStack
    import concourse.tile as tile
    from concourse import bacc, mybir

    f32, f16, i16 = mybir.dt.float32, mybir.dt.float16, mybir.dt.int16
    bf16 = mybir.dt.bfloat16
    Sin = mybir.ActivationFunctionType.Sin
    Copy = mybir.ActivationFunctionType.Copy
    Abs = mybir.ActivationFunctionType.Abs
    mult = mybir.AluOpType.mult
    add = mybir.AluOpType.add
    sub_ = mybir.AluOpType.subtract
    maxop = mybir.AluOpType.max
    ntiles = bs // 128

    nc = bacc.Bacc("TRN2", target_bir_lowering=False, debug=False)
    x_ext = nc.declare_dram_parameter("x16", [bs, PIX], f16, isOutput=False)
    ia_ext = nc.declare_dram_parameter("ia", [bs, PIX], i16, isOutput=False)
    cc_ext = nc.declare_dram_parameter("cc", [bs, rounds * NG], i16, isOutput=False)
    th_ext = nc.declare_dram_parameter("theta", [128, C], f32, isOutput=False)
    out_ext = nc.declare_dram_parameter("out", [bs, OUTW], f16, isOutput=True)

    PI, TWO_PI = math.pi, 2 * math.pi
    MAGIC, INV2PI = 1.5 * 2 ** 23, 1.0 / (2 * math.pi)

    with tile.TileContext(nc) as tc, ExitStack() as ctx:
        cpool = ctx.enter_context(tc.tile_pool(name="const", bufs=1))
        xpool = ctx.enter_context(tc.tile_pool(name="xf", bufs=2))
        ipool = ctx.enter_context(tc.tile_pool(name="ia", bufs=2))
        kpool = ctx.enter_context(tc.tile_pool(name="cc", bufs=2))
        spool = ctx.enter_context(tc.tile_pool(name="sc", bufs=2))
        upool = ctx.enter_context(tc.tile_pool(name="uc", bufs=2))
        vpool = ctx.enter_context(tc.tile_pool(name="v", bufs=2))
        tpool = ctx.enter_context(tc.tile_pool(name="trig", bufs=2))
        wpool = ctx.enter_context(tc.tile_pool(name="we", bufs=2))
        epool = ctx.enter_context(tc.tile_pool(name="expand", bufs=3))
        opool = ctx.enter_context(tc.tile_pool(name="ot", bufs=2))

        pihalf = cpool.tile([128, 1], f32)
        nc.vector.memset(pihalf[:], PI / 2)
        zerob = cpool.tile([128, 1], f32)
        nc.vector.memset(zerob[:], 0.0)

        # Scalar-engine Sin only accepts [-pi, pi]. Range-reduce with the
        # round-to-nearest magic trick: n = (v/2pi + M) - M, -r = 2pi*n - v.
        # Then -sin(v) = Sin(-r) and cos(v) = Sin(pi/2 - |r|); the sin sign
        # flip cancels in sin*sin products and is absorbed into nhst.
        def trig(pool, src, width, tagp, on_act):
            """returns (cv, svN) = (cos(src), -sin(src)), width cols."""
            t1 = pool.tile([128, width], f32, tag=tagp + "t1")
            if on_act:
                nc.scalar.activation(t1[:], src, Copy, bias=MAGIC, scale=INV2PI)
                nc.scalar.activation(t1[:], t1[:], Copy, bias=-MAGIC, scale=1.0)
            else:
                nc.vector.tensor_scalar(t1[:], src, INV2PI, MAGIC, mult, add)
                nc.vector.tensor_scalar(t1[:], t1[:], MAGIC, None, sub_)
            negr = pool.tile([128, width], f16, tag=tagp + "negr")
            nc.vector.scalar_tensor_tensor(negr[:], t1[:], TWO_PI, src, mult, sub_)
            absr = pool.tile([128, width], f16, tag=tagp + "absr")
            nc.vector.tensor_scalar(absr[:], negr[:], -1.0, None, mult)
            nc.vector.tensor_tensor(absr[:], absr[:], negr[:], maxop)  # |r|
            cv = pool.tile([128, width], f16, tag=tagp + "cv")
            svN = pool.tile([128, width], f16, tag=tagp + "svN")
            nc.scalar.activation(svN[:], negr[:], Sin, bias=zerob[:, 0:1])
            nc.scalar.activation(cv[:], absr[:], Sin, bias=pihalf[:, 0:1], scale=-1.0)
            return cv, svN

        # theta coefficients: hct = 0.5*cos(theta), nhst = -0.5*sin(theta)
        th_sb = cpool.tile([128, C], f32)
        nc.sync.dma_start(out=th_sb[:], in_=th_ext[:, :])
        tt1 = cpool.tile([128, C], f32)
        nc.vector.tensor_scalar(tt1[:], th_sb[:], INV2PI, MAGIC, mult, add)
        nc.vector.tensor_scalar(tt1[:], tt1[:], MAGIC, None, sub_)
        tnegr = cpool.tile([128, C], f32)
        nc.vector.scalar_tensor_tensor(tnegr[:], tt1[:], TWO_PI, th_sb[:], mult, sub_)
        nc.vector.tensor_scalar(tt1[:], tnegr[:], -1.0, None, mult)
        nc.vector.tensor_tensor(tt1[:], tt1[:], tnegr[:], maxop)
        cvt = cpool.tile([128, C], f32)
        svNt = cpool.tile([128, C], f32)
        nc.scalar.activation(svNt[:], tnegr[:], Sin, bias=zerob[:, 0:1])
        nc.scalar.activation(cvt[:], tt1[:], Sin, bias=pihalf[:, 0:1], scale=-1.0)
        hcoef = cpool.tile([128, 2 * C], f32)
        nc.vector.tensor_scalar(hcoef[:, 0:C], cvt[:], 0.5, None, mult)
        nc.vector.tensor_scalar(hcoef[:, C:2 * C], svNt[:], 0.5, None, mult)
        hct = hcoef[:, 0:C]        # 0.5*cos(theta)
        nhst = hcoef[:, C:2 * C]   # -0.5*sin(theta) = 0.5*svN

        # pair-major replicated theta tables (one-time, via ACT): unit
        # stride operands let the per-tile products hit the 16-bit 2x mode
        hrep = cpool.tile([128, NPAIR * C], bf16)
        nrep = cpool.tile([128, NPAIR * C], bf16)
        nc.scalar.activation(
            hrep[:].rearrange("p (a b) -> p a b", b=C),
            hct.unsqueeze(1).broadcast_to([128, NPAIR, C]), Copy,
        )
        nc.scalar.activation(
            nrep[:].rearrange("p (a b) -> p a b", b=C),
            nhst.unsqueeze(1).broadcast_to([128, NPAIR, C]), Copy,
        )

        for t in range(ntiles):
            rows = slice(t * 128, (t + 1) * 128)
            ia = ipool.tile([128, PIX], i16)
            xf = xpool.tile([128, PIX], f16)
            if t == 0:
                # split the first tile's DMAs and round-0 scatter into
                # pixel quarters so work starts before the full tile lands
                QX = PIX // 4
                for q in range(4):
                    qs = slice(q * QX, (q + 1) * QX)
                    nc.sync.dma_start(out=ia[:, qs], in_=ia_ext[rows, qs])
                    nc.sync.dma_start(out=xf[:, qs], in_=x_ext[rows, qs])
            else:
                nc.sync.dma_start(out=ia[:], in_=ia_ext[rows, :])
                nc.sync.dma_start(out=xf[:], in_=x_ext[rows, :])
            ct = kpool.tile([128, rounds * NG], i16)
            nc.sync.dma_start(out=ct[:], in_=cc_ext[rows, :])

            # log-doubling scatter rounds; U accumulates (disjoint supports)
            S = spool.tile([128, (rounds + 1) * NG], f16)
            U = upool.tile([128, rounds * NG], f16)
            if t == 0:
                QX = PIX // 4
                s0q = cpool.tile([128, 3 * NG], f16)
                for q in range(4):
                    qs = slice(q * QX, (q + 1) * QX)
                    dstq = S[:, 0:NG] if q == 0 else s0q[:, (q - 1) * NG:q * NG]
                    nc.gpsimd.local_scatter(
                        dstq, xf[:, qs], ia[:, qs],
                        channels=128, num_elems=NG, num_idxs=QX,
                    )
                    if q > 0:
                        nc.vector.tensor_tensor(
                            S[:, 0:NG], S[:, 0:NG],
                            s0q[:, (q - 1) * NG:q * NG], add,
                        )
            else:
                nc.gpsimd.local_scatter(
                    S[:, 0:NG], xf[:], ia[:],
                    channels=128, num_elems=NG, num_idxs=PIX,
                )
            # pure log-doubling: round r scatters from the running union
            # U_{r-1} (ordinals [0, 2^{r-1})) to ordinals [2^{r-1}, 2^r),
            # so capacity(R) = 2^R (R=3 covers multiplicity up to 8)
            V = vpool.tile([128, NG], f16)

            def S_(r):
                return S[:, r * NG:(r + 1) * NG]

            nc.gpsimd.local_scatter(
                S_(1), S_(0), ct[:, 0:NG],
                channels=128, num_elems=NG, num_idxs=NG,
            )
            prev = U[:, 0:NG]
            nc.vector.tensor_tensor(prev, S_(0), S_(1), add)      # U1
            for r in range(2, rounds + 1):
                nc.gpsimd.local_scatter(
                    S_(r), prev, ct[:, (r - 1) * NG:r * NG],
                    channels=128, num_elems=NG, num_idxs=NG,
                )
                dst = V[:] if r == rounds else U[:, (r - 1) * NG:r * NG]
                nc.vector.tensor_tensor(dst, prev, S_(r), add)    # U_r / V
                prev = dst

            cv, sv = trig(tpool, V[:], NG, "g", True)

            # half-split layout: slots [0:460] = x0, [460:920] = x1
            w = wpool.tile([128, NPAIR], f16, tag="w")
            e = wpool.tile([128, NPAIR], f16, tag="e")
            nc.vector.tensor_tensor(w[:], sv[:, 0:NPAIR], sv[:, NPAIR:NG], mult)
            nc.vector.tensor_tensor(e[:], cv[:, 0:NPAIR], cv[:, NPAIR:NG], mult)

            # class expansion: even = (A*hct_c + 0.5) + W*nhst_c, odd = 0.5*E+0.5
            # ACT (non-contending) replicates A and W pair-major; the DVE
            # products are then all-unit-stride 16-bit (2x mode), in-place.
            # Processed in pair-halves so ACT/DVE/DMA pipeline in the drain.
            tev = epool.tile([128, NPAIR * C], bf16, tag="tev")
            tw2 = epool.tile([128, NPAIR * C], bf16, tag="tw2")
            ot = opool.tile([128, OUTW], f16)
            otv = ot[:].rearrange("p (a b) -> p a b", b=2 * C)
            HP = NPAIR // 2
            for h in range(2):
                ph = slice(h * HP, (h + 1) * HP)
                pc = slice(h * HP * C, (h + 1) * HP * C)
                tev3 = tev[:, pc].rearrange("p (a b) -> p a b", b=C)
                tw23 = tw2[:, pc].rearrange("p (a b) -> p a b", b=C)
                A3 = cv[:, ph].unsqueeze(2).broadcast_to([128, HP, C])
                W3 = w[:, ph].unsqueeze(2).broadcast_to([128, HP, C])
                E3 = e[:, ph].unsqueeze(2).broadcast_to([128, HP, C])
                nc.scalar.activation(tev3, A3, Copy)
                nc.vector.tensor_tensor(tev[:, pc], tev[:, pc], hrep[:, pc], mult)
                nc.scalar.activation(tw23, W3, Copy)
                nc.vector.tensor_tensor(tw2[:, pc], tw2[:, pc], nrep[:, pc], mult)
                nc.vector.scalar_tensor_tensor(
                    otv[:, ph, 0:C], tev3, 0.5, tw23, add, add
                )
                nc.scalar.activation(otv[:, ph, C:2 * C], E3, Copy, bias=0.5, scale=0.5)
                nc.sync.dma_start(
                    out=out_ext[rows, h * HP * 2 * C:(h + 1) * HP * 2 * C],
                    in_=ot[:, h * HP * 2 * C:(h + 1) * HP * 2 * C],
                )

    nc.compile()
    return nc


def _prep_scatter_maps(pair_idx):
    """Build round-0 scatter map and log-doubling duplicate maps.

    Slot layout is half-split: x0 of pair p -> slot p, x1 -> slot 460+p.
    Chain round 0 serves ordinal 1, round 1 serves ordinals 2-3 (via the
    running union), round 2 serves 4-7, etc.
    Returns (idxA [B, PIX] i16, chains [T, B, NG] i16, T).
    """
    pidx = pair_idx.reshape(B, NPAIR, 2)
    idx = np.concatenate([pidx[:, :, 0], pidx[:, :, 1]], axis=1).astype(np.int64)
    j = np.arange(NG, dtype=np.int64)[None, :]
    ordk = np.argsort(idx * 1024 + j, axis=1)      # slots sorted by (pixel, slot)
    px_sorted = np.take_along_axis(idx, ordk, axis=1)
    first = np.ones((B, NG), dtype=bool)
    first[:, 1:] = px_sorted[:, 1:] != px_sorted[:, :-1]
    kk = np.broadcast_to(np.arange(NG, dtype=np.int64), (B, NG))
    run_start = np.maximum.accumulate(np.where(first, kk, 0), axis=1)
    o = kk - run_start                              # occurrence ordinal per sorted pos
    maxmult = int(o.max()) + 1
    T = 2
    while (1 << T) < maxmult:                       # capacity(T) = 2^T
        T += 1

    idxA = np.full((B, PIX), -1, np.int16)
    rr, cc = np.nonzero(first)
    idxA[rr, px_sorted[rr, cc]] = ordk[rr, cc]

    # pure doubling: round t (1-based) serves ordinals [2^{t-1}, 2^t)
    # sourcing from ordinal d - 2^{t-1} (available in U_{t-1})
    chains = np.full((T, B, NG), -1, np.int16)
    rr, cc = np.nonzero(o >= 1)
    d = o[rr, cc]
    t_of = np.int64(np.floor(np.log2(d.astype(np.float64)) + 1e-9))
    src_off = np.int64(1) << t_of
    src = ordk[rr, cc - src_off]
    dst = ordk[rr, cc]
    chains[t_of, rr, src] = dst
    return idxA, chains, T


def _get_nc(rounds):
    key = ("nc", rounds)
    if key not in _cache:
        _cache[key] = build_nc(rounds=rounds)
    return _cache[key]


def kernel(x, pair_idx, theta):
    _ensure_path()
    from concourse.bass_utils import run_bass_kernel_spmd

    x16 = np.ascontiguousarray(
        np.asarray(x, dtype=np.float32).reshape(B, PIX).astype(np.float16)
    )
    idxA, chains, T = _prep_scatter_maps(np.asarray(pair_idx))
    nc = _get_nc(T)
    cc = np.ascontiguousarray(
        chains.transpose(1, 0, 2).reshape(B, T * NG)
    )
    thb = np.ascontiguousarray(
        np.tile(np.asarray(theta, dtype=np.float32).reshape(1, C), (128, 1))
    )
    in_maps = [
        {
            "x16": x16[k * BS:(k + 1) * BS],
            "ia": idxA[k * BS:(k + 1) * BS],
            "cc": cc[k * BS:(k + 1) * BS],
            "theta": thb,
        }
        for k in range(NCORES)
    ]
    res = run_bass_kernel_spmd(nc, in_maps, list(range(NCORES))).results
    out = np.concatenate(
        [res[k]["out"].astype(np.float32) for k in range(NCORES)], axis=0
    )
    return out.reshape(B, NG, C)



# revision 7
# speedup vs baseline: 1.0183x; 1.0183x over previous
"""Trainium2 Bass kernel for the fuzzy joint-membership layer.

Math (derived from the reference 2-qubit circuit, verified vs oracle):
  out[b, 2p,   c] = 0.5 + 0.5*cos(theta_c)*cos(x0) - 0.5*sin(theta_c)*sin(x0)*sin(x1)
  out[b, 2p+1, c] = 0.5 + 0.5*cos(x0)*cos(x1)
where x0 = xf[b, pair_idx[b,p,0]], x1 = xf[b, pair_idx[b,p,1]].

Sharding: pure data parallel, batch 4096 -> 8 cores x 512 rows.

Gather strategy (v2, sorted-run fill):
  - host sorts each row's 920 slot requests by pixel; duplicates become
    consecutive runs in the sorted order
  - round 0: gpsimd local_scatter lands x[pix] at the FIRST position of
    its run (map idxA[row, pix] = sorted pos or -1); later run positions
    are zero
  - fill rounds j=0..2: DVE copy_predicated copies position s-2^j -> s
    where host mask m_j[s]=1 (run ordinal of s in [2^j, 2^{j+1})); the
    in-place trailing-shift read only uses lanes whose ordinal < 2^j,
    which this pass never writes, so it is race-free
  - one final gpsimd local_scatter permutes sorted order -> half-split
    slot layout (x0 of pair p -> slot p, x1 -> slot 460+p)
  This replaces the 3 gpsimd chain-scatter rounds (920-wide each) of v1
  with 1 gpsimd permute + 3 cheap DVE predicated copies.

Output: even columns (class-dependent) and the class-INDEPENDENT odd
value are written as uint8 fixed-point (x*253 + 1.25); the host dequants
and replicates the odd value across the 10 classes (pure replication, no
flops). Range reduction (magic round) + Sin + Abs run on ACT; products
and class expansion on DVE.
"""

import math
import numpy as np

B, PIX, NPAIR, C = 4096, 3072, 460, 10
NG = 2 * NPAIR          # 920 gathered values per row
NCORES = 8
BS = B // NCORES        # 512 rows per core
TILES = BS // 128       # 4
GUARD = 8               # leading guard cols in the fill buffer

# u8 fixed-point: stored = clamp(round_or_trunc(253*val + 1.25))
OSCALE = 253.0
OBIAS = 1.25
# host dequant offset (calibrated on HW: cast rounds vs truncates)
DEQ_OFF = -1.25

_cache = {}


def _ensure_path():
    try:
        import concourse  # noqa: F401
    except ImportError:
        import sys
        sys.path.insert(0, "/opt/trn_rl_repo")


def build_nc(bs=BS, rounds=3, exp_mode="B"):
    _ensure_path()
    from contextlib import ExitStack
    import concourse.tile as tile
    from concourse import bacc, mybir

    f32, f16, i16 = mybir.dt.float32, mybir.dt.float16, mybir.dt.int16
    u8 = mybir.dt.uint8
    Sin = mybir.ActivationFunctionType.Sin
    Copy = mybir.ActivationFunctionType.Copy
    Abs = mybir.ActivationFunctionType.Abs
    mult = mybir.AluOpType.mult
    add = mybir.AluOpType.add
    sub_ = mybir.AluOpType.subtract
    maxop = mybir.AluOpType.max
    ntiles = bs // 128

    mpw = 8 + (rounds + 2) * NG   # rounds u8 masks + 8 pad + 920 i16 perm
    nc = bacc.Bacc("TRN2", target_bir_lowering=False, debug=False)
    x_ext = nc.declare_dram_parameter("x16", [bs, PIX], f16, isOutput=False)
    ia_ext = nc.declare_dram_parameter("ia", [bs, PIX], i16, isOutput=False)
    mp_ext = nc.declare_dram_parameter("mp", [bs, mpw], u8, isOutput=False)
    th_ext = nc.declare_dram_parameter("theta", [128, C], f32, isOutput=False)
    oute_ext = nc.declare_dram_parameter("oute", [bs, NPAIR * C], u8, isOutput=True)
    oto_ext = nc.declare_dram_parameter("oto", [bs, NPAIR], u8, isOutput=True)

    PI, TWO_PI = math.pi, 2 * math.pi
    MAGIC, INV2PI = 1.5 * 2 ** 23, 1.0 / (2 * math.pi)
    HSC = 0.5 * OSCALE                   # 126.5
    HB = 0.5 * OSCALE + OBIAS            # 127.75

    with tile.TileContext(nc) as tc, ExitStack() as ctx:
        cpool = ctx.enter_context(tc.tile_pool(name="const", bufs=1))
        xpool = ctx.enter_context(tc.tile_pool(name="xf", bufs=2))
        ipool = ctx.enter_context(tc.tile_pool(name="ia", bufs=2))
        mpool = ctx.enter_context(tc.tile_pool(name="mp", bufs=2))
        fpool = ctx.enter_context(tc.tile_pool(name="fill", bufs=2))
        vpool = ctx.enter_context(tc.tile_pool(name="v", bufs=2))
        tpool = ctx.enter_context(tc.tile_pool(name="trig", bufs=2))
        wpool = ctx.enter_context(tc.tile_pool(name="we", bufs=2))
        epool = ctx.enter_context(tc.tile_pool(name="expand", bufs=2))
        opool = ctx.enter_context(tc.tile_pool(name="ot", bufs=2))

        pihalf = cpool.tile([128, 1], f32)
        nc.vector.memset(pihalf[:], PI / 2)
        zerob = cpool.tile([128, 1], f32)
        nc.vector.memset(zerob[:], 0.0)

        # theta coefficients: hct = HSC*cos(theta), nhst = -HSC*sin(theta)
        th_sb = cpool.tile([128, C], f32)
        nc.sync.dma_start(out=th_sb[:], in_=th_ext[:, :])
        tt1 = cpool.tile([128, C], f32)
        nc.vector.tensor_scalar(tt1[:], th_sb[:], INV2PI, MAGIC, mult, add)
        nc.vector.tensor_scalar(tt1[:], tt1[:], MAGIC, None, sub_)
        tnegr = cpool.tile([128, C], f32)
        nc.vector.scalar_tensor_tensor(tnegr[:], tt1[:], TWO_PI, th_sb[:], mult, sub_)
        nc.vector.tensor_scalar(tt1[:], tnegr[:], -1.0, None, mult)
        nc.vector.tensor_tensor(tt1[:], tt1[:], tnegr[:], maxop)
        cvt = cpool.tile([128, C], f32)
        svNt = cpool.tile([128, C], f32)
        nc.scalar.activation(svNt[:], tnegr[:], Sin, bias=zerob[:, 0:1])
        nc.scalar.activation(cvt[:], tt1[:], Sin, bias=pihalf[:, 0:1], scale=-1.0)
        hcoef = cpool.tile([128, 2 * C], f32)
        nc.vector.tensor_scalar(hcoef[:, 0:C], cvt[:], HSC, None, mult)
        nc.vector.tensor_scalar(hcoef[:, C:2 * C], svNt[:], HSC, None, mult)
        hct = hcoef[:, 0:C]        # HSC*cos(theta)
        nhst = hcoef[:, C:2 * C]   # -HSC*sin(theta)

        # pair-major replicated theta tables (one-time, via ACT)
        hrep = cpool.tile([128, NPAIR * C], f16)
        nrep = cpool.tile([128, NPAIR * C], f16)
        nc.scalar.activation(
            hrep[:].rearrange("p (a b) -> p a b", b=C),
            hct.unsqueeze(1).broadcast_to([128, NPAIR, C]), Copy,
        )
        nc.scalar.activation(
            nrep[:].rearrange("p (a b) -> p a b", b=C),
            nhst.unsqueeze(1).broadcast_to([128, NPAIR, C]), Copy,
        )

        for t in range(ntiles):
            rows = slice(t * 128, (t + 1) * 128)
            xf = xpool.tile([128, PIX], f16)
            ia = ipool.tile([128, PIX], i16)
            nc.sync.dma_start(out=xf[:], in_=x_ext[rows, :])
            nc.sync.dma_start(out=ia[:], in_=ia_ext[rows, :])
            mp = mpool.tile([128, mpw], u8)
            nc.sync.dma_start(out=mp[:], in_=mp_ext[rows, :])

            def mask_(j):
                return mp[:, j * NG:(j + 1) * NG]

            perm = mp[:, rounds * NG + 8:mpw].bitcast(i16)

            # round-0 scatter into sorted-run layout (with guard cols)
            F = fpool.tile([128, GUARD + NG], f16)
            Fw = F[:, GUARD:GUARD + NG]
            nc.gpsimd.local_scatter(
                Fw, xf[:], ia[:],
                channels=128, num_elems=NG, num_idxs=PIX,
            )
            # in-place masked fill: position s (run ordinal in [2^j,2^{j+1}))
            # copies from s - 2^j; sources have ordinal < 2^j and are never
            # written in the same pass
            for j in range(rounds):
                sh = 1 << j
                nc.vector.copy_predicated(
                    Fw, mask_(j), F[:, GUARD - sh:GUARD - sh + NG],
                )
            # permute sorted order -> half-split slots
            V = vpool.tile([128, NG], f16)
            nc.gpsimd.local_scatter(
                V[:], Fw, perm,
                channels=128, num_elems=NG, num_idxs=NG,
            )

            # trig: magic range-reduction on ACT, Sin/Abs on ACT, one DVE stt
            t1 = tpool.tile([128, NG], f32, tag="t1")
            nc.scalar.activation(t1[:], V[:], Copy, bias=MAGIC, scale=INV2PI)
            nc.scalar.activation(t1[:], t1[:], Copy, bias=-MAGIC, scale=1.0)
            negr = tpool.tile([128, NG], f16, tag="negr")
            nc.vector.scalar_tensor_tensor(negr[:], t1[:], TWO_PI, V[:], mult, sub_)
            absr = tpool.tile([128, NG], f16, tag="absr")
            nc.scalar.activation(absr[:], negr[:], Abs, bias=zerob[:, 0:1])
            cv = tpool.tile([128, NG], f16, tag="cv")
            sv = tpool.tile([128, NG], f16, tag="sv")
            nc.scalar.activation(sv[:], negr[:], Sin, bias=zerob[:, 0:1])
            nc.scalar.activation(cv[:], absr[:], Sin, bias=pihalf[:, 0:1], scale=-1.0)

            # half-split layout: slots [0:460] = x0, [460:920] = x1
            w = wpool.tile([128, NPAIR], f16, tag="w")
            e = wpool.tile([128, NPAIR], f16, tag="e")
            nc.vector.tensor_tensor(w[:], sv[:, 0:NPAIR], sv[:, NPAIR:NG], mult)
            nc.vector.tensor_tensor(e[:], cv[:, 0:NPAIR], cv[:, NPAIR:NG], mult)

            # class expansion: even_u8 = (A*hct + HB) + W*nhst
            tev = epool.tile([128, NPAIR * C], f16, tag="tev")
            tw2 = epool.tile([128, NPAIR * C], f16, tag="tw2")
            ote = opool.tile([128, NPAIR * C], u8, tag="ote")
            oto = opool.tile([128, NPAIR], u8, tag="oto")
            A3 = cv[:, 0:NPAIR].unsqueeze(2).broadcast_to([128, NPAIR, C])
            W3 = w[:].unsqueeze(2).broadcast_to([128, NPAIR, C])
            tev3 = tev[:].rearrange("p (a b) -> p a b", b=C)
            tw23 = tw2[:].rearrange("p (a b) -> p a b", b=C)
            hrep3 = hrep[:].rearrange("p (a b) -> p a b", b=C)
            nrep3 = nrep[:].rearrange("p (a b) -> p a b", b=C)
            if exp_mode == "B":
                # DVE broadcast-operand products
                nc.vector.tensor_tensor(tev3, A3, hrep3, mult)
                nc.vector.tensor_tensor(tw23, W3, nrep3, mult)
            else:
                # ACT replicates, DVE multiplies unit-stride
                nc.scalar.activation(tev3, A3, Copy)
                nc.vector.tensor_tensor(tev[:], tev[:], hrep[:], mult)
                nc.scalar.activation(tw23, W3, Copy)
                nc.vector.tensor_tensor(tw2[:], tw2[:], nrep[:], mult)
            nc.vector.scalar_tensor_tensor(ote[:], tev[:], HB, tw2[:], add, add)
            # odd value (class-independent): HSC*E + HB
            nc.scalar.activation(oto[:], e[:], Copy, bias=HB, scale=HSC)
            nc.sync.dma_start(out=oute_ext[rows, :], in_=ote[:])
            nc.sync.dma_start(out=oto_ext[rows, :], in_=oto[:])

    nc.compile()
    return nc


def _prep_maps(pair_idx, rounds=3):
    """Build round-0 scatter map (pixel -> first sorted pos), fill masks,
    and the sorted->half-split permutation.

    Returns (idxA [B,PIX] i16, mp [B,MPW] u8, rounds).
    """
    pidx = pair_idx.reshape(B, NPAIR, 2)
    idx = np.concatenate([pidx[:, :, 0], pidx[:, :, 1]], axis=1).astype(np.int64)
    j = np.arange(NG, dtype=np.int64)[None, :]
    ordk = np.argsort(idx * 1024 + j, axis=1)      # sorted by (pixel, slot)
    px_sorted = np.take_along_axis(idx, ordk, axis=1)
    first = np.ones((B, NG), dtype=bool)
    first[:, 1:] = px_sorted[:, 1:] != px_sorted[:, :-1]
    kk = np.broadcast_to(np.arange(NG, dtype=np.int64), (B, NG))
    run_start = np.maximum.accumulate(np.where(first, kk, 0), axis=1)
    o = kk - run_start                              # run ordinal per sorted pos
    maxmult = int(o.max()) + 1
    while (1 << rounds) < maxmult:
        rounds += 1

    idxA = np.full((B, PIX), -1, np.int16)
    rr, cc = np.nonzero(first)
    idxA[rr, px_sorted[rr, cc]] = cc.astype(np.int16)

    masks = np.zeros((rounds, B, NG), np.uint8)
    for jr in range(rounds):
        masks[jr] = ((o >= (1 << jr)) & (o < (2 << jr))).astype(np.uint8)
    perm = ordk.astype(np.int16)                    # sorted pos -> final slot

    mp = np.zeros((B, 8 + (rounds + 2) * NG), np.uint8)
    mp[:, 0:rounds * NG] = masks.transpose(1, 0, 2).reshape(B, rounds * NG)
    mp[:, rounds * NG + 8:] = perm.view(np.uint8).reshape(B, 2 * NG)
    return idxA, mp, rounds


def _get_nc(rounds):
    key = ("nc", rounds)
    if key not in _cache:
        _cache[key] = build_nc(rounds=rounds)
    return _cache[key]


def kernel(x, pair_idx, theta):
    _ensure_path()
    from concourse.bass_utils import run_bass_kernel_spmd

    x16 = np.ascontiguousarray(
        np.asarray(x, dtype=np.float32).reshape(B, PIX).astype(np.float16)
    )
    idxA, mp, rounds = _prep_maps(np.asarray(pair_idx))
    nc = _get_nc(rounds)
    thb = np.ascontiguousarray(
        np.tile(np.asarray(theta, dtype=np.float32).reshape(1, C), (128, 1))
    )
    in_maps = [
        {
            "x16": x16[k * BS:(k + 1) * BS],
            "ia": idxA[k * BS:(k + 1) * BS],
            "mp": mp[k * BS:(k + 1) * BS],
            "theta": thb,
        }
        for k in range(NCORES)
    ]
    res = run_bass_kernel_spmd(nc, in_maps, list(range(NCORES))).results
    out = np.empty((B, NG, C), np.float32)
    oe = out.reshape(B, NPAIR, 2, C)
    inv = np.float32(1.0 / OSCALE)
    off = np.float32(DEQ_OFF)
    for k in range(NCORES):
        rows = slice(k * BS, (k + 1) * BS)
        ev = (res[k]["oute"].astype(np.float32) + off) * inv
        od = (res[k]["oto"].astype(np.float32) + off) * inv
        oe[rows, :, 0, :] = ev.reshape(BS, NPAIR, C)
        oe[rows, :, 1, :] = od[:, :, None]
    return out


# revision 15
# speedup vs baseline: 1.0820x; 1.0625x over previous
"""Trainium2 Bass kernel for the fuzzy joint-membership layer.

Math (derived from the reference 2-qubit circuit, verified vs oracle):
  out[b, 2p,   c] = 0.5 + 0.5*cos(theta_c)*cos(x0) - 0.5*sin(theta_c)*sin(x0)*sin(x1)
  out[b, 2p+1, c] = 0.5 + 0.5*cos(x0)*cos(x1)
where x0 = xf[b, pair_idx[b,p,0]], x1 = xf[b, pair_idx[b,p,1]].

Sharding: pure data parallel, batch 4096 -> 8 cores x 512 rows.

Gather strategy (v2, sorted-run fill):
  - host sorts each row's 920 slot requests by pixel; duplicates become
    consecutive runs in the sorted order
  - round 0: gpsimd local_scatter lands x[pix] at the FIRST position of
    its run (map idxA[row, pix] = sorted pos or -1); later run positions
    are zero
  - fill rounds j=0..2: DVE copy_predicated copies position s-2^j -> s
    where host mask m_j[s]=1 (run ordinal of s in [2^j, 2^{j+1})); the
    in-place trailing-shift read only uses lanes whose ordinal < 2^j,
    which this pass never writes, so it is race-free
  - one final gpsimd local_scatter permutes sorted order -> half-split
    slot layout (x0 of pair p -> slot p, x1 -> slot 460+p)
  This replaces the 3 gpsimd chain-scatter rounds (920-wide each) of v1
  with 1 gpsimd permute + 3 cheap DVE predicated copies.

Output: even columns (class-dependent) and the class-INDEPENDENT odd
value are written as uint8 fixed-point (x*253 + 1.25); the host dequants
and replicates the odd value across the 10 classes (pure replication, no
flops). Range reduction (magic round) + Sin + Abs run on ACT; products
and class expansion on DVE.
"""

import math
import numpy as np

B, PIX, NPAIR, C = 4096, 3072, 460, 10
NG = 2 * NPAIR          # 920 gathered values per row
NCORES = 8
BS = B // NCORES        # 512 rows per core
TILES = BS // 128       # 4
GUARD = 8               # leading guard cols in the fill buffer



_cache = {}


def _ensure_path():
    try:
        import concourse  # noqa: F401
    except ImportError:
        import sys
        sys.path.insert(0, "/opt/trn_rl_repo")


def build_nc(bs=BS, rounds=3, exp_mode="B"):
    _ensure_path()
    from contextlib import ExitStack
    import concourse.tile as tile
    from concourse import bacc, mybir

    f32, f16, i16 = mybir.dt.float32, mybir.dt.float16, mybir.dt.int16
    u8 = mybir.dt.uint8
    Sin = mybir.ActivationFunctionType.Sin
    Copy = mybir.ActivationFunctionType.Copy
    Abs = mybir.ActivationFunctionType.Abs
    mult = mybir.AluOpType.mult
    add = mybir.AluOpType.add
    sub_ = mybir.AluOpType.subtract
    maxop = mybir.AluOpType.max
    ntiles = bs // 128

    mpw = 8 + (rounds + 2) * NG   # rounds u8 masks + 8 pad + 920 i16 perm
    nc = bacc.Bacc("TRN2", target_bir_lowering=False, debug=False)
    x_ext = nc.declare_dram_parameter("x16", [bs, PIX], f16, isOutput=False)
    ia_ext = nc.declare_dram_parameter("ia", [bs, PIX], i16, isOutput=False)
    mp_ext = nc.declare_dram_parameter("mp", [bs, mpw], u8, isOutput=False)
    th_ext = nc.declare_dram_parameter("theta", [128, C], f32, isOutput=False)
    oute_ext = nc.declare_dram_parameter("oute", [bs, NPAIR * C], f16, isOutput=True)
    oto_ext = nc.declare_dram_parameter("oto", [bs, NPAIR], f16, isOutput=True)

    PI, TWO_PI = math.pi, 2 * math.pi
    MAGIC, INV2PI = 1.5 * 2 ** 23, 1.0 / (2 * math.pi)

    with tile.TileContext(nc) as tc, ExitStack() as ctx:
        cpool = ctx.enter_context(tc.tile_pool(name="const", bufs=1))
        xpool = ctx.enter_context(tc.tile_pool(name="xf", bufs=2))
        ipool = ctx.enter_context(tc.tile_pool(name="ia", bufs=2))
        mpool = ctx.enter_context(tc.tile_pool(name="mp", bufs=2))
        fpool = ctx.enter_context(tc.tile_pool(name="fill", bufs=2))
        vpool = ctx.enter_context(tc.tile_pool(name="v", bufs=2))
        tpool = ctx.enter_context(tc.tile_pool(name="trig", bufs=2))
        wpool = ctx.enter_context(tc.tile_pool(name="we", bufs=2))
        epool = ctx.enter_context(tc.tile_pool(name="expand", bufs=2))
        opool = ctx.enter_context(tc.tile_pool(name="ot", bufs=2))

        pihalf = cpool.tile([128, 1], f32)
        nc.vector.memset(pihalf[:], PI / 2)
        zerob = cpool.tile([128, 1], f32)
        nc.vector.memset(zerob[:], 0.0)

        # theta coefficients: hct = HSC*cos(theta), nhst = -HSC*sin(theta)
        th_sb = cpool.tile([128, C], f32)
        nc.sync.dma_start(out=th_sb[:], in_=th_ext[:, :])
        tt1 = cpool.tile([128, C], f32)
        nc.vector.tensor_scalar(tt1[:], th_sb[:], INV2PI, MAGIC, mult, add)
        nc.vector.tensor_scalar(tt1[:], tt1[:], MAGIC, None, sub_)
        tnegr = cpool.tile([128, C], f32)
        nc.vector.scalar_tensor_tensor(tnegr[:], tt1[:], TWO_PI, th_sb[:], mult, sub_)
        nc.vector.tensor_scalar(tt1[:], tnegr[:], -1.0, None, mult)
        nc.vector.tensor_tensor(tt1[:], tt1[:], tnegr[:], maxop)
        cvt = cpool.tile([128, C], f32)
        svNt = cpool.tile([128, C], f32)
        nc.scalar.activation(svNt[:], tnegr[:], Sin, bias=zerob[:, 0:1])
        nc.scalar.activation(cvt[:], tt1[:], Sin, bias=pihalf[:, 0:1], scale=-1.0)
        hcoef = cpool.tile([128, 2 * C], f32)
        nc.vector.tensor_scalar(hcoef[:, 0:C], cvt[:], 0.5, None, mult)
        nc.vector.tensor_scalar(hcoef[:, C:2 * C], svNt[:], 0.5, None, mult)
        hct = hcoef[:, 0:C]        # 0.5*cos(theta)
        nhst = hcoef[:, C:2 * C]   # -0.5*sin(theta)

        # class-major replicated theta tables (one-time, via ACT):
        # hrep[p, c, a] = hct[p, c] for all pairs a
        hrep = cpool.tile([128, C * NPAIR], f16)
        nrep = cpool.tile([128, C * NPAIR], f16)
        nc.scalar.activation(
            hrep[:].rearrange("p (c a) -> p c a", a=NPAIR),
            hct.unsqueeze(2).broadcast_to([128, C, NPAIR]), Copy,
        )
        nc.scalar.activation(
            nrep[:].rearrange("p (c a) -> p c a", a=NPAIR),
            nhst.unsqueeze(2).broadcast_to([128, C, NPAIR]), Copy,
        )

        for t in range(ntiles):
            rows = slice(t * 128, (t + 1) * 128)
            xf = xpool.tile([128, PIX], f16)
            ia = ipool.tile([128, PIX], i16)
            nc.sync.dma_start(out=xf[:], in_=x_ext[rows, :])
            nc.sync.dma_start(out=ia[:], in_=ia_ext[rows, :])
            mp = mpool.tile([128, mpw], u8)
            nc.sync.dma_start(out=mp[:], in_=mp_ext[rows, :])

            def mask_(j):
                return mp[:, j * NG:(j + 1) * NG]

            perm = mp[:, rounds * NG + 8:mpw].bitcast(i16)

            # round-0 scatter into sorted-run layout (with guard cols)
            F = fpool.tile([128, GUARD + NG], f16)
            Fw = F[:, GUARD:GUARD + NG]
            nc.gpsimd.local_scatter(
                Fw, xf[:], ia[:],
                channels=128, num_elems=NG, num_idxs=PIX,
            )
            # in-place masked fill: position s (run ordinal in [2^j,2^{j+1}))
            # copies from s - 2^j; sources have ordinal < 2^j and are never
            # written in the same pass
            for j in range(rounds):
                sh = 1 << j
                nc.vector.copy_predicated(
                    Fw, mask_(j), F[:, GUARD - sh:GUARD - sh + NG],
                )
            # permute sorted order -> half-split slots
            V = vpool.tile([128, NG], f16)
            nc.gpsimd.local_scatter(
                V[:], Fw, perm,
                channels=128, num_elems=NG, num_idxs=NG,
            )

            # trig: magic range-reduction on ACT, Sin/Abs on ACT, one DVE stt
            t1 = tpool.tile([128, NG], f32, tag="t1")
            nc.scalar.activation(t1[:], V[:], Copy, bias=MAGIC, scale=INV2PI)
            nc.scalar.activation(t1[:], t1[:], Copy, bias=-MAGIC, scale=1.0)
            negr = tpool.tile([128, NG], f16, tag="negr")
            nc.vector.scalar_tensor_tensor(negr[:], t1[:], TWO_PI, V[:], mult, sub_)
            absr = tpool.tile([128, NG], f16, tag="absr")
            nc.scalar.activation(absr[:], negr[:], Abs, bias=zerob[:, 0:1])
            cv = tpool.tile([128, NG], f16, tag="cv")
            sv = tpool.tile([128, NG], f16, tag="sv")
            nc.scalar.activation(sv[:], negr[:], Sin, bias=zerob[:, 0:1])
            nc.scalar.activation(cv[:], absr[:], Sin, bias=pihalf[:, 0:1], scale=-1.0)

            # half-split layout: slots [0:460] = x0, [460:920] = x1
            w = wpool.tile([128, NPAIR], f16, tag="w")
            e = wpool.tile([128, NPAIR], f16, tag="e")
            nc.vector.tensor_tensor(w[:], sv[:, 0:NPAIR], sv[:, NPAIR:NG], mult)
            nc.vector.tensor_tensor(e[:], cv[:, 0:NPAIR], cv[:, NPAIR:NG], mult)

            # class expansion (CLASS-MAJOR [c, pair]): even = A*hct + W*nhst
            # (host adds the 0.5 and transposes to pair-major). A and W are
            # replicated via SBUF->SBUF DMA (stride-0 middle dim, contiguous
            # fastest dim); products + add are unit-stride f16 on DVE (2x).
            tev = epool.tile([128, C * NPAIR], f16, tag="tev")
            tw2 = epool.tile([128, C * NPAIR], f16, tag="tw2")
            ote = opool.tile([128, C * NPAIR], f16, tag="ote")
            oto = opool.tile([128, NPAIR], f16, tag="oto")
            A3 = cv[:, 0:NPAIR].unsqueeze(1).broadcast_to([128, C, NPAIR])
            W3 = w[:].unsqueeze(1).broadcast_to([128, C, NPAIR])
            tev3 = tev[:].rearrange("p (c a) -> p c a", a=NPAIR)
            tw23 = tw2[:].rearrange("p (c a) -> p c a", a=NPAIR)
            nc.sync.dma_start(out=tev3, in_=A3)
            nc.vector.tensor_tensor(tev[:], tev[:], hrep[:], mult)
            nc.sync.dma_start(out=tw23, in_=W3)
            nc.vector.tensor_tensor(tw2[:], tw2[:], nrep[:], mult)
            nc.vector.tensor_tensor(ote[:], tev[:], tw2[:], add)
            # odd value (class-independent): 0.5*E (host adds the 0.5)
            nc.scalar.activation(oto[:], e[:], Copy, bias=0.0, scale=0.5)
            nc.sync.dma_start(out=oute_ext[rows, :], in_=ote[:])
            nc.sync.dma_start(out=oto_ext[rows, :], in_=oto[:])

    nc.compile()
    return nc


def _prep_maps(pair_idx, rounds=3):
    """Build round-0 scatter map (pixel -> first sorted pos), fill masks,
    and the sorted->half-split permutation.

    Returns (idxA [B,PIX] i16, mp [B,MPW] u8, rounds).
    """
    pidx = pair_idx.reshape(B, NPAIR, 2)
    idx = np.concatenate([pidx[:, :, 0], pidx[:, :, 1]], axis=1).astype(np.int64)
    j = np.arange(NG, dtype=np.int64)[None, :]
    ordk = np.argsort(idx * 1024 + j, axis=1)      # sorted by (pixel, slot)
    px_sorted = np.take_along_axis(idx, ordk, axis=1)
    first = np.ones((B, NG), dtype=bool)
    first[:, 1:] = px_sorted[:, 1:] != px_sorted[:, :-1]
    kk = np.broadcast_to(np.arange(NG, dtype=np.int64), (B, NG))
    run_start = np.maximum.accumulate(np.where(first, kk, 0), axis=1)
    o = kk - run_start                              # run ordinal per sorted pos
    maxmult = int(o.max()) + 1
    while (1 << rounds) < maxmult:
        rounds += 1

    idxA = np.full((B, PIX), -1, np.int16)
    rr, cc = np.nonzero(first)
    idxA[rr, px_sorted[rr, cc]] = cc.astype(np.int16)

    masks = np.zeros((rounds, B, NG), np.uint8)
    for jr in range(rounds):
        masks[jr] = ((o >= (1 << jr)) & (o < (2 << jr))).astype(np.uint8)
    perm = ordk.astype(np.int16)                    # sorted pos -> final slot

    mp = np.zeros((B, 8 + (rounds + 2) * NG), np.uint8)
    mp[:, 0:rounds * NG] = masks.transpose(1, 0, 2).reshape(B, rounds * NG)
    mp[:, rounds * NG + 8:] = perm.view(np.uint8).reshape(B, 2 * NG)
    return idxA, mp, rounds


def _get_nc(rounds):
    key = ("nc", rounds)
    if key not in _cache:
        _cache[key] = build_nc(rounds=rounds)
    return _cache[key]


def kernel(x, pair_idx, theta):
    _ensure_path()
    from concourse.bass_utils import run_bass_kernel_spmd

    x16 = np.ascontiguousarray(
        np.asarray(x, dtype=np.float32).reshape(B, PIX).astype(np.float16)
    )
    idxA, mp, rounds = _prep_maps(np.asarray(pair_idx))
    nc = _get_nc(rounds)
    thb = np.ascontiguousarray(
        np.tile(np.asarray(theta, dtype=np.float32).reshape(1, C), (128, 1))
    )
    in_maps = [
        {
            "x16": x16[k * BS:(k + 1) * BS],
            "ia": idxA[k * BS:(k + 1) * BS],
            "mp": mp[k * BS:(k + 1) * BS],
            "theta": thb,
        }
        for k in range(NCORES)
    ]
    res = run_bass_kernel_spmd(nc, in_maps, list(range(NCORES))).results
    out = np.empty((B, NG, C), np.float32)
    oe = out.reshape(B, NPAIR, 2, C)
    for k in range(NCORES):
        rows = slice(k * BS, (k + 1) * BS)
        ev = res[k]["oute"].astype(np.float32) + np.float32(0.5)
        od = res[k]["oto"].astype(np.float32) + np.float32(0.5)
        oe[rows, :, 0, :] = ev.reshape(BS, C, NPAIR).transpose(0, 2, 1)
        oe[rows, :, 1, :] = od[:, :, None]
    return out


# revision 17
# speedup vs baseline: 1.2367x; 1.1430x over previous
"""Trainium2 Bass kernel for the fuzzy joint-membership layer.

Math (derived from the reference 2-qubit circuit, verified vs oracle):
  out[b, 2p,   c] = 0.5 + 0.5*cos(theta_c)*cos(x0) - 0.5*sin(theta_c)*sin(x0)*sin(x1)
  out[b, 2p+1, c] = 0.5 + 0.5*cos(x0)*cos(x1)
where x0 = xf[b, pair_idx[b,p,0]], x1 = xf[b, pair_idx[b,p,1]].

Sharding: pure data parallel, batch 4096 -> 8 cores x 512 rows.

Gather strategy (v2, sorted-run fill):
  - host sorts each row's 920 slot requests by pixel; duplicates become
    consecutive runs in the sorted order
  - round 0: gpsimd local_scatter lands x[pix] at the FIRST position of
    its run (map idxA[row, pix] = sorted pos or -1); later run positions
    are zero
  - fill rounds j=0..2: DVE copy_predicated copies position s-2^j -> s
    where host mask m_j[s]=1 (run ordinal of s in [2^j, 2^{j+1})); the
    in-place trailing-shift read only uses lanes whose ordinal < 2^j,
    which this pass never writes, so it is race-free
  - one final gpsimd local_scatter permutes sorted order -> half-split
    slot layout (x0 of pair p -> slot p, x1 -> slot 460+p)
  This replaces the 3 gpsimd chain-scatter rounds (920-wide each) of v1
  with 1 gpsimd permute + 3 cheap DVE predicated copies.

Output: even columns (class-dependent) and the class-INDEPENDENT odd
value are written as uint8 fixed-point (x*253 + 1.25); the host dequants
and replicates the odd value across the 10 classes (pure replication, no
flops). Range reduction (magic round) + Sin + Abs run on ACT; products
and class expansion on DVE.
"""

import math
import numpy as np

B, PIX, NPAIR, C = 4096, 3072, 460, 10
NG = 2 * NPAIR          # 920 gathered values per row
NCORES = 8
BS = B // NCORES        # 512 rows per core
TILES = BS // 128       # 4
GUARD = 8               # leading guard cols in the fill buffer



_cache = {}


def _ensure_path():
    try:
        import concourse  # noqa: F401
    except ImportError:
        import sys
        sys.path.insert(0, "/opt/trn_rl_repo")


def build_nc(bs=BS, rounds=3, exp_mode="B"):
    _ensure_path()
    from contextlib import ExitStack
    import concourse.tile as tile
    from concourse import bacc, mybir

    f32, f16, i16 = mybir.dt.float32, mybir.dt.float16, mybir.dt.int16
    u8 = mybir.dt.uint8
    Sin = mybir.ActivationFunctionType.Sin
    Copy = mybir.ActivationFunctionType.Copy
    Abs = mybir.ActivationFunctionType.Abs
    mult = mybir.AluOpType.mult
    add = mybir.AluOpType.add
    sub_ = mybir.AluOpType.subtract
    maxop = mybir.AluOpType.max
    ntiles = bs // 128

    mpw = 8 + (rounds + 2) * NG   # rounds u8 masks + 8 pad + 920 i16 perm
    nc = bacc.Bacc("TRN2", target_bir_lowering=False, debug=False)
    x_ext = nc.declare_dram_parameter("x16", [bs, PIX], f16, isOutput=False)
    ia_ext = nc.declare_dram_parameter("ia", [bs, PIX], i16, isOutput=False)
    mp_ext = nc.declare_dram_parameter("mp", [bs, mpw], u8, isOutput=False)
    th_ext = nc.declare_dram_parameter("theta", [128, C], f32, isOutput=False)
    oute_ext = nc.declare_dram_parameter("oute", [bs, NPAIR * C], f16, isOutput=True)
    oto_ext = nc.declare_dram_parameter("oto", [bs, NPAIR], f16, isOutput=True)

    PI, TWO_PI = math.pi, 2 * math.pi
    MAGIC, INV2PI = 1.5 * 2 ** 23, 1.0 / (2 * math.pi)

    with tile.TileContext(nc) as tc, ExitStack() as ctx:
        cpool = ctx.enter_context(tc.tile_pool(name="const", bufs=1))
        xpool = ctx.enter_context(tc.tile_pool(name="xf", bufs=2))
        ipool = ctx.enter_context(tc.tile_pool(name="ia", bufs=2))
        mpool = ctx.enter_context(tc.tile_pool(name="mp", bufs=2))
        fpool = ctx.enter_context(tc.tile_pool(name="fill", bufs=2))
        vpool = ctx.enter_context(tc.tile_pool(name="v", bufs=2))
        tpool = ctx.enter_context(tc.tile_pool(name="trig", bufs=2))
        wpool = ctx.enter_context(tc.tile_pool(name="we", bufs=2))
        epool = ctx.enter_context(tc.tile_pool(name="expand", bufs=2))
        opool = ctx.enter_context(tc.tile_pool(name="ot", bufs=2))

        pihalf = cpool.tile([128, 1], f32)
        nc.vector.memset(pihalf[:], PI / 2)
        zerob = cpool.tile([128, 1], f32)
        nc.vector.memset(zerob[:], 0.0)

        # theta coefficients: hct = HSC*cos(theta), nhst = -HSC*sin(theta)
        th_sb = cpool.tile([128, C], f32)
        nc.sync.dma_start(out=th_sb[:], in_=th_ext[:, :])
        tt1 = cpool.tile([128, C], f32)
        nc.vector.tensor_scalar(tt1[:], th_sb[:], INV2PI, MAGIC, mult, add)
        nc.vector.tensor_scalar(tt1[:], tt1[:], MAGIC, None, sub_)
        tnegr = cpool.tile([128, C], f32)
        nc.vector.scalar_tensor_tensor(tnegr[:], tt1[:], TWO_PI, th_sb[:], mult, sub_)
        nc.vector.tensor_scalar(tt1[:], tnegr[:], -1.0, None, mult)
        nc.vector.tensor_tensor(tt1[:], tt1[:], tnegr[:], maxop)
        cvt = cpool.tile([128, C], f32)
        svNt = cpool.tile([128, C], f32)
        nc.scalar.activation(svNt[:], tnegr[:], Sin, bias=zerob[:, 0:1])
        nc.scalar.activation(cvt[:], tt1[:], Sin, bias=pihalf[:, 0:1], scale=-1.0)
        hcoef = cpool.tile([128, 2 * C], f32)
        nc.vector.tensor_scalar(hcoef[:, 0:C], cvt[:], 0.5, None, mult)
        nc.vector.tensor_scalar(hcoef[:, C:2 * C], svNt[:], 0.5, None, mult)
        hct = hcoef[:, 0:C]        # 0.5*cos(theta)
        nhst = hcoef[:, C:2 * C]   # -0.5*sin(theta)

        # class-major replicated theta tables (one-time, via ACT):
        # hrep[p, c, a] = hct[p, c] for all pairs a
        hrep = cpool.tile([128, C * NPAIR], f16)
        nrep = cpool.tile([128, C * NPAIR], f16)
        nc.scalar.activation(
            hrep[:].rearrange("p (c a) -> p c a", a=NPAIR),
            hct.unsqueeze(2).broadcast_to([128, C, NPAIR]), Copy,
        )
        nc.scalar.activation(
            nrep[:].rearrange("p (c a) -> p c a", a=NPAIR),
            nhst.unsqueeze(2).broadcast_to([128, C, NPAIR]), Copy,
        )

        for t in range(ntiles):
            rows = slice(t * 128, (t + 1) * 128)
            xf = xpool.tile([128, PIX], f16)
            ia = ipool.tile([128, PIX], i16)
            if t == 0:
                # halve the first tile's loads + scatter so GpSimd starts
                # as soon as the first half lands (cuts the pipeline ramp)
                HX = PIX // 2
                nc.sync.dma_start(out=xf[:, 0:HX], in_=x_ext[rows, 0:HX])
                nc.sync.dma_start(out=ia[:, 0:HX], in_=ia_ext[rows, 0:HX])
                nc.sync.dma_start(out=xf[:, HX:PIX], in_=x_ext[rows, HX:PIX])
                nc.sync.dma_start(out=ia[:, HX:PIX], in_=ia_ext[rows, HX:PIX])
            else:
                nc.sync.dma_start(out=xf[:], in_=x_ext[rows, :])
                nc.sync.dma_start(out=ia[:], in_=ia_ext[rows, :])
            mp = mpool.tile([128, mpw], u8)
            nc.sync.dma_start(out=mp[:], in_=mp_ext[rows, :])

            def mask_(j):
                return mp[:, j * NG:(j + 1) * NG]

            perm = mp[:, rounds * NG + 8:mpw].bitcast(i16)

            # round-0 scatter into sorted-run layout (with guard cols)
            F = fpool.tile([128, GUARD + NG], f16)
            Fw = F[:, GUARD:GUARD + NG]
            if t == 0:
                HX = PIX // 2
                F2 = fpool.tile([128, NG], f16, tag="f2")
                nc.gpsimd.local_scatter(
                    Fw, xf[:, 0:HX], ia[:, 0:HX],
                    channels=128, num_elems=NG, num_idxs=HX,
                )
                nc.gpsimd.local_scatter(
                    F2[:], xf[:, HX:PIX], ia[:, HX:PIX],
                    channels=128, num_elems=NG, num_idxs=HX,
                )
                nc.vector.tensor_tensor(Fw, Fw, F2[:], add)
            else:
                nc.gpsimd.local_scatter(
                    Fw, xf[:], ia[:],
                    channels=128, num_elems=NG, num_idxs=PIX,
                )
            # in-place masked fill: position s (run ordinal in [2^j,2^{j+1}))
            # copies from s - 2^j; sources have ordinal < 2^j and are never
            # written in the same pass
            for j in range(rounds):
                sh = 1 << j
                nc.vector.copy_predicated(
                    Fw, mask_(j), F[:, GUARD - sh:GUARD - sh + NG],
                )
            # permute sorted order -> half-split slots
            V = vpool.tile([128, NG], f16)
            nc.gpsimd.local_scatter(
                V[:], Fw, perm,
                channels=128, num_elems=NG, num_idxs=NG,
            )

            # trig: magic range-reduction on ACT, Sin/Abs on ACT, one DVE stt
            t1 = tpool.tile([128, NG], f32, tag="t1")
            nc.scalar.activation(t1[:], V[:], Copy, bias=MAGIC, scale=INV2PI)
            nc.scalar.activation(t1[:], t1[:], Copy, bias=-MAGIC, scale=1.0)
            negr = tpool.tile([128, NG], f16, tag="negr")
            nc.vector.scalar_tensor_tensor(negr[:], t1[:], TWO_PI, V[:], mult, sub_)
            absr = tpool.tile([128, NG], f16, tag="absr")
            nc.scalar.activation(absr[:], negr[:], Abs, bias=zerob[:, 0:1])
            cv = tpool.tile([128, NG], f16, tag="cv")
            sv = tpool.tile([128, NG], f16, tag="sv")
            nc.scalar.activation(sv[:], negr[:], Sin, bias=zerob[:, 0:1])
            nc.scalar.activation(cv[:], absr[:], Sin, bias=pihalf[:, 0:1], scale=-1.0)

            # half-split layout: slots [0:460] = x0, [460:920] = x1
            w = wpool.tile([128, NPAIR], f16, tag="w")
            e = wpool.tile([128, NPAIR], f16, tag="e")
            nc.vector.tensor_tensor(w[:], sv[:, 0:NPAIR], sv[:, NPAIR:NG], mult)
            nc.vector.tensor_tensor(e[:], cv[:, 0:NPAIR], cv[:, NPAIR:NG], mult)

            # class expansion (CLASS-MAJOR [c, pair]): even = A*hct + W*nhst
            # (host adds the 0.5 and transposes to pair-major). Broadcast
            # operands have stride-0 on the MIDDLE dim and unit-stride
            # fastest dim, so DVE still runs the 16-bit 2x mode — no
            # replication pass needed. Last tile runs in class-halves so
            # DVE/DMA pipeline in the drain.
            tw2 = epool.tile([128, C * NPAIR], f16, tag="tw2")
            ote = opool.tile([128, C * NPAIR], f16, tag="ote")
            oto = opool.tile([128, NPAIR], f16, tag="oto")
            A3 = cv[:, 0:NPAIR].unsqueeze(1).broadcast_to([128, C, NPAIR])
            W3 = w[:].unsqueeze(1).broadcast_to([128, C, NPAIR])
            ote3 = ote[:].rearrange("p (c a) -> p c a", a=NPAIR)
            tw23 = tw2[:].rearrange("p (c a) -> p c a", a=NPAIR)
            hrep3 = hrep[:].rearrange("p (c a) -> p c a", a=NPAIR)
            nrep3 = nrep[:].rearrange("p (c a) -> p c a", a=NPAIR)
            nhalves = 2 if t == ntiles - 1 else 1
            HC = C // nhalves
            for h in range(nhalves):
                cs = slice(h * HC, (h + 1) * HC)
                fs = slice(h * HC * NPAIR, (h + 1) * HC * NPAIR)
                nc.vector.tensor_tensor(ote3[:, cs], A3[:, cs], hrep3[:, cs], mult)
                nc.vector.tensor_tensor(tw23[:, cs], W3[:, cs], nrep3[:, cs], mult)
                nc.vector.tensor_tensor(ote[:, fs], ote[:, fs], tw2[:, fs], add)
                nc.sync.dma_start(out=oute_ext[rows, fs], in_=ote[:, fs])
            # odd value (class-independent): 0.5*E (host adds the 0.5)
            nc.scalar.activation(oto[:], e[:], Copy, bias=0.0, scale=0.5)
            nc.sync.dma_start(out=oto_ext[rows, :], in_=oto[:])

    nc.compile()
    return nc


def _prep_maps(pair_idx, rounds=3):
    """Build round-0 scatter map (pixel -> first sorted pos), fill masks,
    and the sorted->half-split permutation.

    Returns (idxA [B,PIX] i16, mp [B,MPW] u8, rounds).
    """
    pidx = pair_idx.reshape(B, NPAIR, 2)
    idx = np.concatenate([pidx[:, :, 0], pidx[:, :, 1]], axis=1).astype(np.int64)
    j = np.arange(NG, dtype=np.int64)[None, :]
    ordk = np.argsort(idx * 1024 + j, axis=1)      # sorted by (pixel, slot)
    px_sorted = np.take_along_axis(idx, ordk, axis=1)
    first = np.ones((B, NG), dtype=bool)
    first[:, 1:] = px_sorted[:, 1:] != px_sorted[:, :-1]
    kk = np.broadcast_to(np.arange(NG, dtype=np.int64), (B, NG))
    run_start = np.maximum.accumulate(np.where(first, kk, 0), axis=1)
    o = kk - run_start                              # run ordinal per sorted pos
    maxmult = int(o.max()) + 1
    while (1 << rounds) < maxmult:
        rounds += 1

    idxA = np.full((B, PIX), -1, np.int16)
    rr, cc = np.nonzero(first)
    idxA[rr, px_sorted[rr, cc]] = cc.astype(np.int16)

    masks = np.zeros((rounds, B, NG), np.uint8)
    for jr in range(rounds):
        masks[jr] = ((o >= (1 << jr)) & (o < (2 << jr))).astype(np.uint8)
    perm = ordk.astype(np.int16)                    # sorted pos -> final slot

    mp = np.zeros((B, 8 + (rounds + 2) * NG), np.uint8)
    mp[:, 0:rounds * NG] = masks.transpose(1, 0, 2).reshape(B, rounds * NG)
    mp[:, rounds * NG + 8:] = perm.view(np.uint8).reshape(B, 2 * NG)
    return idxA, mp, rounds


def _get_nc(rounds):
    key = ("nc", rounds)
    if key not in _cache:
        _cache[key] = build_nc(rounds=rounds)
    return _cache[key]


def kernel(x, pair_idx, theta):
    _ensure_path()
    from concourse.bass_utils import run_bass_kernel_spmd

    x16 = np.ascontiguousarray(
        np.asarray(x, dtype=np.float32).reshape(B, PIX).astype(np.float16)
    )
    idxA, mp, rounds = _prep_maps(np.asarray(pair_idx))
    nc = _get_nc(rounds)
    thb = np.ascontiguousarray(
        np.tile(np.asarray(theta, dtype=np.float32).reshape(1, C), (128, 1))
    )
    in_maps = [
        {
            "x16": x16[k * BS:(k + 1) * BS],
            "ia": idxA[k * BS:(k + 1) * BS],
            "mp": mp[k * BS:(k + 1) * BS],
            "theta": thb,
        }
        for k in range(NCORES)
    ]
    res = run_bass_kernel_spmd(nc, in_maps, list(range(NCORES))).results
    out = np.empty((B, NG, C), np.float32)
    oe = out.reshape(B, NPAIR, 2, C)
    for k in range(NCORES):
        rows = slice(k * BS, (k + 1) * BS)
        ev = res[k]["oute"].astype(np.float32) + np.float32(0.5)
        od = res[k]["oto"].astype(np.float32) + np.float32(0.5)
        oe[rows, :, 0, :] = ev.reshape(BS, C, NPAIR).transpose(0, 2, 1)
        oe[rows, :, 1, :] = od[:, :, None]
    return out
